# revision 39
# baseline (speedup 1.0000x reference)
"""Sparse (masked) multi-head attention on 8 Trainium2 NeuronCores.

Problem: nodes [2,2048,512], edge_mask [2,2048,2048] (bool),
q/kv/o linear layers with H=8 heads of DH=64.

Sharding: batch x head-group.  Core c handles batch b = c//4 and head group
g = c%4 (heads 2g, 2g+1 = inner columns g*128:(g+1)*128).  Each core
computes its two heads' attention over the full sequence plus its partial
contribution to the output projection; the host sums the 4 partials per
batch and adds bo.

v2 schedule: the ScalarE exp stream (64 x [128,1024], ~70us) is the
bottleneck, co-critical with the PE matmul stream (~164k cycles).  So:
  - ScalarE runs ONLY the exps; biases/casts/copies go to DVE/Pool.
  - k-bias is dropped entirely (adds q_i.bk to every logit of row i:
    softmax-invariant).
  - minimal prologue: qH0+kH0 projections right after nT lands; first
    exp at ~11us.  kH1/qH1 are emitted into spare PE slots at jb0/jb1
    (psum slots num1/num0 before the AV accumulators claim them);
    v-projection is interleaved per-jb into the i-half-0 loop.
  - mask DMA is 16 per-jb transfers behind nT; jb arrival (1.4us) beats
    consumption (2.1us/jb).
  - i-half-0's output projection aliases the sim psum slots and is spread
    through i-half-1's loop; its out-DMA overlaps the loop.

Per-core dataflow (all matmuls bf16 inputs, fp32 PSUM accumulation):
  qT/kT [dh=128, N]  = wq_sliceT @ nodesT (+bq via DVE)  (dh on partitions)
  v     [N, dh=128]  = nodesT.T @ wv_slice
  per head h: simT[j,i] = kTz_h.T @ qT   (j on partitions; kTz zero-padded
              to K=128 so the PE array never runs half-idle -> no HAM clamp)
              PT = exp(simT * DH**-0.5)   (ScalarE, free scale, bf16 out)
              PT *= maskT                 (VectorE, bf16 2x mode)
              numT[0:64,i] / den[64,i] = [v_h | 1].T @ PT  (ones col -> denom)
              attnT_h = numT * recip(den)  (recip + partition_broadcast)
  out[i,:] += attnT.T @ wo_slice          (contraction over both heads)
"""
import numpy as np
import ml_dtypes

import concourse.bass as bass
import concourse.bacc as bacc
import concourse.tile as tile
from concourse import mybir
from concourse.bass_utils import run_bass_kernel_spmd
from bass_rust import add_dep_helper

B, N, DIM = 2, 2048, 512
H, DH = 8, 64
INNER = H * DH
SCALE = DH ** -0.5
NCORES = 8
HEADS_PER_CORE = 2
HG = 128            # inner columns per core (2 heads x 64)
NJB = N // 128      # 16 j-blocks
NC_DIM = DIM // 128  # 4 contraction chunks over DIM
NH = N // 2          # i-half extent

BF16 = mybir.dt.bfloat16
F32 = mybir.dt.float32
ts = bass.ts
ds = bass.ds


def _build():
    nc = bacc.Bacc(monotonic_sem_count=0)
    nT_d = nc.declare_dram_parameter("nodesT", [DIM, N], BF16, isOutput=False)
    maskT_d = nc.declare_dram_parameter("maskT", [N, N], BF16, isOutput=False)
    wq_d = nc.declare_dram_parameter("wq_s", [DIM, HG], BF16, isOutput=False)
    wk_d = nc.declare_dram_parameter("wk_s", [DIM, HG], BF16, isOutput=False)
    wv_d = nc.declare_dram_parameter("wv_s", [DIM, HG], BF16, isOutput=False)
    wo_d = nc.declare_dram_parameter("wo_s", [HG, DIM], BF16, isOutput=False)
    bq_d = nc.declare_dram_parameter("bq_s", [HG, 1], F32, isOutput=False)
    # p-major output: out_pm[p, ib, m] = out[ib*128 + p, m].  Each
    # partition's rows land contiguously, so the output DMA descriptors
    # are 4KB instead of scattered 1KB (the host undoes the transpose).
    out_d = nc.declare_dram_parameter("out", [128, NJB, DIM], BF16, isOutput=True)

    with tile.TileContext(nc) as tc:
        with (
            tc.tile_pool(name="persist", bufs=1) as persist,
            tc.tile_pool(name="ptp", bufs=10) as ptp,
            tc.tile_pool(name="denp", bufs=1) as denp,
            tc.tile_pool(name="outp", bufs=2) as outp,
            # PSUM: 8 banks.  psA = {sim0, sim1} (2 banks each; also host
            # q/k projections, v-proj and o-proj transients), psB = {num0,
            # num1} (2 banks each; also warm-up, kH1/qH1 projections).
            tc.tile_pool(name="psA", bufs=1, space="PSUM") as psA,
            tc.tile_pool(name="psB", bufs=1, space="PSUM") as psB,
        ):
            # ---- input DMA.  Trigger (descriptor-gen) instructions cost
            # ~0.7us each on their host queue, and the sync-hosted queue
            # moves data at roughly half rate — so nT rides the gpsimd
            # HWDGE (fast, free early), k/v/o weights the scalar HWDGE,
            # and the mask is split across both behind the last nT chunk.
            wq = persist.tile([128, NC_DIM, HG], BF16)
            nc.scalar.dma_start(
                out=wq[:], in_=wq_d.rearrange("(c p) m -> p c m", p=128)
            )
            wk = persist.tile([128, NC_DIM, HG], BF16)
            nc.scalar.dma_start(
                out=wk[:], in_=wk_d.rearrange("(c p) m -> p c m", p=128)
            )
            nT = persist.tile([128, NC_DIM, N], BF16)
            nT_r = nT_d.rearrange("(c p) n -> p c n", p=128)
            nt_dma = nc.scalar.dma_start(out=nT[:], in_=nT_r[:])
            nt_dmas = [nt_dma]
            bq = persist.tile([HG, 1], F32)
            nc.scalar.dma_start(out=bq[:], in_=bq_d[:])
            wv = persist.tile([128, NC_DIM, HG], BF16)
            nc.scalar.dma_start(
                out=wv[:], in_=wv_d.rearrange("(c p) m -> p c m", p=128)
            )
            wo = persist.tile([HG, DIM], BF16)
            nc.scalar.dma_start(out=wo[:], in_=wo_d[:])

            # ---- Pool prologue: zero-fills while DMA streams ----
            wrm_src = persist.tile([128, 512], BF16)
            nc.gpsimd.memset(wrm_src[:], 0.0)
            # kTz[:, h, :]: head h's dh rows at their original partitions,
            # the other head's rows zero — sim matmuls contract over all
            # 128 partitions (K=64 would idle half the PE array and trip
            # the HAM clamp).
            kTz = persist.tile([128, 2, N], BF16)
            nc.gpsimd.memset(kTz[:], 0.0)
            # v rows [j, dh] with a ones column appended per head (cols
            # 0:64 = head0 v, col 64 = 1, cols 65:129 = head1 v, col 129
            # = 1); the ones column makes the AV matmul emit the softmax
            # denominator as row 64 of the accumulator.
            v_sb = persist.tile([128, NJB, 130], BF16)
            nc.gpsimd.memset(v_sb[:, :, 64:65], 1.0)
            nc.gpsimd.memset(v_sb[:, :, 129:130], 1.0)
            # first mask half rides the gpsimd HWDGE behind the memsets;
            # the transfers themselves wait for the last nT chunk.  The
            # second half is triggered from the scalar queue after the
            # q-bias (emitted below) so it cannot delay the first exp.
            maskT = persist.tile([128, NJB, N], BF16)
            maskT_r = maskT_d.rearrange("(g p) i -> p g i", p=128)
            for jb in range(NJB // 2):
                d = nc.gpsimd.dma_start(
                    out=maskT[:, jb, :], in_=maskT_r[:, jb, :]
                )
                add_dep_helper(d.ins, nt_dmas[-1].ins, reason="mask after nT")
            # second mask half rides the (otherwise idle, slower) sync
            # queue: 4MB at ~half rate still lands well before jb8 is
            # consumed.
            for jb in range(NJB // 2, NJB):
                d = nc.sync.dma_start(
                    out=maskT[:, jb, :], in_=maskT_r[:, jb, :]
                )
                add_dep_helper(d.ins, nt_dmas[-1].ins, reason="mask after nT")
            # dummy partition_broadcast: loads the Q7 broadcast library on
            # the Pool engine during the prologue (~7us) so the mid-loop
            # broadcasts don't pay the swap.  Nothing but DGE triggers run
            # on Pool afterwards, so the library stays resident.
            bc_dummy_i = persist.tile([1, 32], F32)
            nc.vector.memset(bc_dummy_i[:], 1.0)
            bc_dummy_o = persist.tile([2, 32], F32)
            nc.gpsimd.partition_broadcast(bc_dummy_o[:], bc_dummy_i[:])

            # ---- PE warm-up: dummy matmuls while input DMA streams, so
            # the PE pstate ramps to max before the real projections ----
            wrm_ps = psB.tile([128, 512], F32, tag="num0")
            for i in range(18):
                nc.tensor.matmul(
                    wrm_ps[:], lhsT=wrm_src[:, 0:128], rhs=wrm_src[:],
                    start=(i == 0), stop=(i == 17),
                )
            wrm_out = persist.tile([128, 512], BF16)
            nc.vector.tensor_copy(wrm_out[:], wrm_ps[:])

            qT = persist.tile([128, N], BF16)
            attnT = persist.tile([128, N], BF16)

            def qproj0():
                pps = psA.tile([128, NH], F32, tag="sim0", name="qp0")
                for isl in range(2):
                    for c in range(NC_DIM):
                        nc.tensor.matmul(
                            pps[:, ts(isl, 512)],
                            lhsT=wq[:, c, :],
                            rhs=nT[:, c, ts(isl, 512)],
                            start=(c == 0),
                            stop=(c == NC_DIM - 1),
                        )
                # ScalarE is idle until the first exp — bias lives there.
                nc.scalar.activation(
                    out=qT[:, 0:NH], in_=pps[:],
                    func=mybir.ActivationFunctionType.Identity, bias=bq[:],
                )

            def kproj_mm(half, pool, tag):
                kps = pool.tile([128, NH], F32, tag=tag, name=f"kp{half}")
                for isl in range(2):
                    for c in range(NC_DIM):
                        nc.tensor.matmul(
                            kps[:, ts(isl, 512)],
                            lhsT=wk[:, c, :],
                            rhs=nT[:, c, ts(half * 2 + isl, 512)],
                            start=(c == 0),
                            stop=(c == NC_DIM - 1),
                        )
                return kps

            def kproj_cast(half, kps, copy):
                # no k-bias: it only shifts each query row's logits
                # uniformly, which softmax cancels.
                copy(kTz[0:64, 0, ts(half, NH)], kps[0:64, :])
                copy(kTz[64:128, 1, ts(half, NH)], kps[64:128, :])

            sc_copy = lambda out_, in_: nc.scalar.copy(out=out_, in_=in_)

            # prologue projections: only what the first sims need.  The
            # bias rides ScalarE, the casts DVE — both idle pre-loop.
            qproj0()
            kps0 = kproj_mm(0, psA, "sim1")
            kproj_cast(0, kps0, nc.vector.tensor_copy)

            def vproj(jb):
                vps = psA.tile([128, HG], F32, tag=f"sim{jb % 2}",
                               name=f"vp{jb}")
                for c in range(NC_DIM):
                    nc.tensor.matmul(
                        vps[:],
                        lhsT=nT[:, c, ts(jb, 128)],
                        rhs=wv[:, c, :],
                        start=(c == 0),
                        stop=(c == NC_DIM - 1),
                    )
                nc.vector.tensor_copy(
                    v_sb[:, jb, 0:130].rearrange("p (h c) -> p h c", h=2)[
                        :, :, 0:64
                    ],
                    vps[:].rearrange("p (h c) -> p h c", h=2),
                )

            def sim_exp_mask(jb, h, io):
                sps = psA.tile([128, NH], F32, tag=f"sim{h}", name="sps")
                for isl in range(2):
                    nc.tensor.matmul(
                        sps[:, ts(isl, 512)],
                        lhsT=kTz[:, h, ts(jb, 128)],
                        rhs=qT[:, ds(io + isl * 512, 512)],
                        start=True,
                        stop=True,
                    )
                pt = ptp.tile([128, NH], BF16, tag="pt")
                nc.scalar.activation(
                    out=pt[:],
                    in_=sps[:],
                    func=mybir.ActivationFunctionType.Exp,
                    scale=SCALE,
                )
                nc.vector.tensor_mul(pt[:], pt[:], maskT[:, jb, ds(io, NH)])
                return pt

            def av(jb, h, pt, npss):
                for isl in range(2):
                    nc.tensor.matmul(
                        npss[h][:, ts(isl, 512)],
                        lhsT=v_sb[:, jb, ts(h, 65)],
                        rhs=pt[:, ts(isl, 512)],
                        start=(jb == 0),
                        stop=(jb == NJB - 1),
                    )

            def norm_recip(ihalf, h, npss):
                if ihalf == 0:
                    # copy the accumulator to SBUF right away so the PSUM
                    # slot frees for i-half-1's AVs.
                    nsb = denp.tile([65, NH], F32, tag=f"nsb{h}")
                    nc.vector.tensor_copy(nsb[:], npss[h][:])
                    num_src = nsb[0:64, :]
                    # DVE silently reads partition 0 when its input AP
                    # starts at partition 64 — move the den row to a
                    # partition-0 tile (DMA on the idle gpsimd queue)
                    # before the reciprocal.
                    den1 = denp.tile([1, NH], F32, tag=f"den1{h}")
                    nc.gpsimd.dma_start(out=den1[:], in_=nsb[64:65, :])
                    den_src = den1[:]
                else:
                    # tail: nothing follows — read the accumulator in
                    # place; ScalarE is idle after the last exp.
                    num_src = npss[h][0:64, :]
                    den1 = denp.tile([1, NH], F32, tag=f"den1{h}")
                    nc.scalar.copy(out=den1[:], in_=npss[h][64:65, :])
                    den_src = den1[:]
                rec1 = denp.tile([1, NH], F32, tag=f"rec1{h}")
                nc.vector.reciprocal_approx_fast(out=rec1[:], in_=den_src)
                return num_src, rec1

            def norm_mul(h, io, num_src, rec1, copy):
                # replicate the reciprocal row across partitions 0:64 on
                # the Pool engine (library pre-loaded in the prologue) —
                # touches no PSUM slot, so the exp/sim stream never waits.
                rec = denp.tile([64, NH], F32, tag=f"rec{h}")
                nc.gpsimd.partition_broadcast(rec[:], rec1[:])
                nc.vector.tensor_mul(
                    attnT[ts(h, 64), ds(io, NH)], num_src, rec[:]
                )

            out_r = out_d
            osb = {}

            def oproj_ib(ib, copy):
                grp, k = ib // 4, ib % 4
                if k == 0:
                    osb[grp] = outp.tile([128, 4, DIM], BF16, tag="osb",
                                         name=f"osb{grp}")
                ops = psA.tile([128, DIM], F32, tag=f"sim{ib % 2}",
                               name=f"op{ib}")
                nc.tensor.matmul(
                    ops[:], lhsT=attnT[:, ts(ib, 128)], rhs=wo[:],
                    start=True, stop=True,
                )
                copy(osb[grp][:, k, :], ops[:])
                if k == 3:
                    eng = nc.scalar if grp == 3 else nc.gpsimd
                    eng.dma_start(
                        out=out_r[:, ts(grp, 4), :], in_=osb[grp][:]
                    )

            # ---- i-half 0: v-projection and the remaining q/k halves are
            # threaded into the loop's spare PE slots.  psB tag claim order
            # is warm -> kH1/qH1 -> AV accumulators, matching emission. ----
            io = 0
            vproj(0)
            pt00 = sim_exp_mask(0, 0, io)
            pt01 = sim_exp_mask(0, 1, io)
            vproj(1)
            pt10 = sim_exp_mask(1, 0, io)
            pt11 = sim_exp_mask(1, 1, io)
            # kH1 into psB num1, qH1 into psB num0 — before the AV
            # accumulators claim those slots.
            kps1 = kproj_mm(1, psB, "num1")
            kproj_cast(1, kps1, nc.vector.tensor_copy)
            qps1 = psB.tile([128, NH], F32, tag="num0", name="qp1")
            for isl in range(2):
                for c in range(NC_DIM):
                    nc.tensor.matmul(
                        qps1[:, ts(isl, 512)],
                        lhsT=wq[:, c, :],
                        rhs=nT[:, c, ts(2 + isl, 512)],
                        start=(c == 0),
                        stop=(c == NC_DIM - 1),
                    )
            nc.vector.tensor_scalar_add(
                out=qT[:, ts(1, NH)], in0=qps1[:], scalar1=bq[:]
            )
            # AV accumulators claim the num slots now.
            npss0 = {
                0: psB.tile([65, NH], F32, tag="num0", name="nps00"),
                1: psB.tile([65, NH], F32, tag="num1", name="nps01"),
            }
            av(0, 0, pt00, npss0)
            av(0, 1, pt01, npss0)
            av(1, 0, pt10, npss0)
            av(1, 1, pt11, npss0)
            for jb in range(2, NJB):
                vproj(jb)
                for h in range(HEADS_PER_CORE):
                    pt = sim_exp_mask(jb, h, io)
                    av(jb, h, pt, npss0)
            norms0 = [norm_recip(0, h, npss0) for h in range(HEADS_PER_CORE)]

            # ---- i-half 1; i-half-0's norm-muls and o-projection are
            # spread through it so their PSUM aliasing (sim tags) costs at
            # most one slot-rotation bubble each ----
            io = NH
            npss1 = {
                0: psB.tile([65, NH], F32, tag="num0", name="nps10"),
                1: psB.tile([65, NH], F32, tag="num1", name="nps11"),
            }
            for jb in range(NJB):
                for h in range(HEADS_PER_CORE):
                    pt = sim_exp_mask(jb, h, io)
                    av(jb, h, pt, npss1)
                if 2 <= jb <= 3:
                    num_src, rec1 = norms0[jb - 2]
                    norm_mul(jb - 2, 0, num_src, rec1, nc.vector.tensor_copy)
                if 5 <= jb <= 12:
                    oproj_ib(jb - 5, nc.vector.tensor_copy)
            wrm2 = psA.tile([128, 512], F32, tag="sim0", name="wrm2")
            for i in range(14):
                nc.tensor.matmul(
                    wrm2[:], lhsT=wrm_src[:, 0:128], rhs=wrm_src[:],
                    start=(i == 0), stop=(i == 13),
                )
            for h in range(HEADS_PER_CORE):
                num_src, rec1 = norm_recip(1, h, npss1)
                norm_mul(h, io, num_src, rec1, sc_copy)
            for ib in range(8, 16):
                oproj_ib(ib, sc_copy)

    # Bacc.compile runs generate_event_semaphores, which splits multi-sem
    # waits down to the 1-wait-per-instruction limit walrus enforces.
    nc.compile()

    # Bacc's dce_regs leaves the (unread) engine-preamble register writes
    # behind at this kernel size, with deferred reg_id=-1 — walrus then
    # fails "Reg has not been allocated yet".  Nothing reads them, so any
    # valid unique per-engine id works.
    from collections import defaultdict

    next_id = defaultdict(lambda: 8)
    for a in nc.m.functions[0].allocations:
        if type(a).__name__ == "Register" and a.reg_id == -1:
            a.reg_id = next_id[str(a.engine)]
            next_id[str(a.engine)] += 1
    return nc


_NC_CACHE = None


def _get_nc():
    global _NC_CACHE
    if _NC_CACHE is None:
        _NC_CACHE = _build()
    return _NC_CACHE


def _prep_in_maps(nodes, edge_mask, wq, bq, wkv, bkv, wo, bo):
    bf16 = ml_dtypes.bfloat16
    wk_full, wv_full = wkv[:, :INNER], wkv[:, INNER:]
    per_batch = []
    for b in range(B):
        per_batch.append(
            (
                np.ascontiguousarray(nodes[b].T).astype(bf16),
                np.ascontiguousarray(edge_mask[b].T).astype(bf16),
            )
        )
    in_maps = []
    for core in range(NCORES):
        b, g = core // 4, core % 4
        cs = slice(g * HG, (g + 1) * HG)
        nT_b, maskT_b = per_batch[b]
        in_maps.append(
            {
                "nodesT": nT_b,
                "maskT": maskT_b,
                "wq_s": np.ascontiguousarray(wq[:, cs]).astype(bf16),
                "wk_s": np.ascontiguousarray(wk_full[:, cs]).astype(bf16),
                "wv_s": np.ascontiguousarray(wv_full[:, cs]).astype(bf16),
                "wo_s": np.ascontiguousarray(wo[cs, :]).astype(bf16),
                "bq_s": np.ascontiguousarray(bq[cs]).reshape(HG, 1).astype(np.float32),
            }
        )
    return in_maps


def kernel(nodes, edge_mask, wq, bq, wkv, bkv, wo, bo, _trace=False, _trace_kwargs=None):
    nodes = np.asarray(nodes, dtype=np.float32)
    edge_mask = np.asarray(edge_mask)
    wq = np.asarray(wq, dtype=np.float32)
    bq = np.asarray(bq, dtype=np.float32)
    wkv = np.asarray(wkv, dtype=np.float32)
    bkv = np.asarray(bkv, dtype=np.float32)
    wo = np.asarray(wo, dtype=np.float32)
    bo = np.asarray(bo, dtype=np.float32)

    nc = _get_nc()
    in_maps = _prep_in_maps(nodes, edge_mask, wq, bq, wkv, bkv, wo, bo)
    kw = {}
    if _trace:
        kw = dict(trace=True, **(_trace_kwargs or {}))
    res = run_bass_kernel_spmd(nc, in_maps, list(range(NCORES)), **kw)
    out = np.zeros((B, N, DIM), np.float32)
    for core in range(NCORES):
        o = res.results[core]["out"].astype(np.float32)   # [128, NJB, DIM]
        out[core // 4] += o.transpose(1, 0, 2).reshape(N, DIM)
    # v-bias shifts each head's attention output by exactly bv (softmax
    # weights sum to 1), so its output contribution is the constant bv @ wo.
    bv_full = bkv[INNER:]
    out += (bv_full @ wo + bo)[None, None, :]
    if _trace:
        return out, res
    return out


# revision 40
# speedup vs baseline: 1.0241x; 1.0241x over previous
"""Sparse (masked) multi-head attention on 8 Trainium2 NeuronCores.

Problem: nodes [2,2048,512], edge_mask [2,2048,2048] (bool),
q/kv/o linear layers with H=8 heads of DH=64.

Sharding: batch x head-group.  Core c handles batch b = c//4 and head group
g = c%4 (heads 2g, 2g+1 = inner columns g*128:(g+1)*128).  Each core
computes its two heads' attention over the full sequence plus its partial
contribution to the output projection; the host sums the 4 partials per
batch and adds bo.

v2 schedule: the ScalarE exp stream (64 x [128,1024], ~70us) is the
bottleneck, co-critical with the PE matmul stream (~164k cycles).  So:
  - ScalarE runs ONLY the exps; biases/casts/copies go to DVE/Pool.
  - k-bias is dropped entirely (adds q_i.bk to every logit of row i:
    softmax-invariant).
  - minimal prologue: qH0+kH0 projections right after nT lands; first
    exp at ~11us.  kH1/qH1 are emitted into spare PE slots at jb0/jb1
    (psum slots num1/num0 before the AV accumulators claim them);
    v-projection is interleaved per-jb into the i-half-0 loop.
  - mask DMA is 16 per-jb transfers behind nT; jb arrival (1.4us) beats
    consumption (2.1us/jb).
  - i-half-0's output projection aliases the sim psum slots and is spread
    through i-half-1's loop; its out-DMA overlaps the loop.

Per-core dataflow (all matmuls bf16 inputs, fp32 PSUM accumulation):
  qT/kT [dh=128, N]  = wq_sliceT @ nodesT (+bq via DVE)  (dh on partitions)
  v     [N, dh=128]  = nodesT.T @ wv_slice
  per head h: simT[j,i] = kTz_h.T @ qT   (j on partitions; kTz zero-padded
              to K=128 so the PE array never runs half-idle -> no HAM clamp)
              PT = exp(simT * DH**-0.5)   (ScalarE, free scale, bf16 out)
              PT *= maskT                 (VectorE, bf16 2x mode)
              numT[0:64,i] / den[64,i] = [v_h | 1].T @ PT  (ones col -> denom)
              attnT_h = numT * recip(den)  (recip + partition_broadcast)
  out[i,:] += attnT.T @ wo_slice          (contraction over both heads)
"""
import numpy as np
import ml_dtypes

import concourse.bass as bass
import concourse.bacc as bacc
import concourse.tile as tile
from concourse import mybir
from concourse.bass_utils import run_bass_kernel_spmd
from bass_rust import add_dep_helper

B, N, DIM = 2, 2048, 512
H, DH = 8, 64
INNER = H * DH
SCALE = DH ** -0.5
NCORES = 8
HEADS_PER_CORE = 2
HG = 128            # inner columns per core (2 heads x 64)
NJB = N // 128      # 16 j-blocks
NC_DIM = DIM // 128  # 4 contraction chunks over DIM
NH = N // 2          # i-half extent

BF16 = mybir.dt.bfloat16
F32 = mybir.dt.float32
ts = bass.ts
ds = bass.ds


def _build():
    nc = bacc.Bacc(monotonic_sem_count=0)
    nT_d = nc.declare_dram_parameter("nodesT", [DIM, N], BF16, isOutput=False)
    maskT_d = nc.declare_dram_parameter("maskT", [N, N], BF16, isOutput=False)
    wq_d = nc.declare_dram_parameter("wq_s", [DIM, HG], BF16, isOutput=False)
    wk_d = nc.declare_dram_parameter("wk_s", [DIM, HG], BF16, isOutput=False)
    wv_d = nc.declare_dram_parameter("wv_s", [DIM, HG], BF16, isOutput=False)
    wo_d = nc.declare_dram_parameter("wo_s", [HG, DIM], BF16, isOutput=False)
    bq_d = nc.declare_dram_parameter("bq_s", [HG, 1], F32, isOutput=False)
    # p-major output: out_pm[p, ib, m] = out[ib*128 + p, m].  Each
    # partition's rows land contiguously, so the output DMA descriptors
    # are 4KB instead of scattered 1KB (the host undoes the transpose).
    out_d = nc.declare_dram_parameter("out", [128, NJB, DIM], BF16, isOutput=True)

    with tile.TileContext(nc) as tc:
        with (
            tc.tile_pool(name="persist", bufs=1) as persist,
            tc.tile_pool(name="ptp", bufs=10) as ptp,
            tc.tile_pool(name="denp", bufs=1) as denp,
            tc.tile_pool(name="outp", bufs=2) as outp,
            # PSUM: 8 banks.  psA = {sim0, sim1} (2 banks each; also host
            # q/k projections, v-proj and o-proj transients), psB = {num0,
            # num1} (2 banks each; also warm-up, kH1/qH1 projections).
            tc.tile_pool(name="psA", bufs=1, space="PSUM") as psA,
            tc.tile_pool(name="psB", bufs=1, space="PSUM") as psB,
        ):
            # ---- input DMA.  Trigger (descriptor-gen) instructions cost
            # ~0.7us each on their host queue, and the sync-hosted queue
            # moves data at roughly half rate — so nT rides the gpsimd
            # HWDGE (fast, free early), k/v/o weights the scalar HWDGE,
            # and the mask is split across both behind the last nT chunk.
            nT = persist.tile([128, NC_DIM, N], BF16)
            nT_r = nT_d.rearrange("(c p) n -> p c n", p=128)
            nt_dma = nc.scalar.dma_start(out=nT[:], in_=nT_r[:])
            nt_dmas = [nt_dma]
            wq = persist.tile([128, NC_DIM, HG], BF16)
            nc.scalar.dma_start(
                out=wq[:], in_=wq_d.rearrange("(c p) m -> p c m", p=128)
            )
            wk = persist.tile([128, NC_DIM, HG], BF16)
            nc.scalar.dma_start(
                out=wk[:], in_=wk_d.rearrange("(c p) m -> p c m", p=128)
            )
            bq = persist.tile([HG, 1], F32)
            nc.scalar.dma_start(out=bq[:], in_=bq_d[:])
            wv = persist.tile([128, NC_DIM, HG], BF16)
            nc.scalar.dma_start(
                out=wv[:], in_=wv_d.rearrange("(c p) m -> p c m", p=128)
            )
            wo = persist.tile([HG, DIM], BF16)
            nc.scalar.dma_start(out=wo[:], in_=wo_d[:])

            # ---- Pool prologue: zero-fills while DMA streams ----
            wrm_src = persist.tile([128, 512], BF16)
            nc.gpsimd.memset(wrm_src[:], 0.0)
            # kTz[:, h, :]: head h's dh rows at their original partitions,
            # the other head's rows zero — sim matmuls contract over all
            # 128 partitions (K=64 would idle half the PE array and trip
            # the HAM clamp).
            kTz = persist.tile([128, 2, N], BF16)
            nc.gpsimd.memset(kTz[:], 0.0)
            # v rows [j, dh] with a ones column appended per head (cols
            # 0:64 = head0 v, col 64 = 1, cols 65:129 = head1 v, col 129
            # = 1); the ones column makes the AV matmul emit the softmax
            # denominator as row 64 of the accumulator.
            v_sb = persist.tile([128, NJB, 130], BF16)
            nc.gpsimd.memset(v_sb[:, :, 64:65], 1.0)
            nc.gpsimd.memset(v_sb[:, :, 129:130], 1.0)
            # first mask half rides the gpsimd HWDGE behind the memsets;
            # the transfers themselves wait for the last nT chunk.  The
            # second half is triggered from the scalar queue after the
            # q-bias (emitted below) so it cannot delay the first exp.
            maskT = persist.tile([128, NJB, N], BF16)
            maskT_r = maskT_d.rearrange("(g p) i -> p g i", p=128)
            for jb in range(NJB // 2):
                d = nc.gpsimd.dma_start(
                    out=maskT[:, jb, :], in_=maskT_r[:, jb, :]
                )
                add_dep_helper(d.ins, nt_dmas[-1].ins, reason="mask after nT")
            # second mask half rides the (otherwise idle, slower) sync
            # queue: 4MB at ~half rate still lands well before jb8 is
            # consumed.
            for jb in range(NJB // 2, NJB):
                d = nc.sync.dma_start(
                    out=maskT[:, jb, :], in_=maskT_r[:, jb, :]
                )
                add_dep_helper(d.ins, nt_dmas[-1].ins, reason="mask after nT")
            # dummy partition_broadcast: loads the Q7 broadcast library on
            # the Pool engine during the prologue (~7us) so the mid-loop
            # broadcasts don't pay the swap.  Nothing but DGE triggers run
            # on Pool afterwards, so the library stays resident.
            bc_dummy_i = persist.tile([1, 32], F32)
            nc.vector.memset(bc_dummy_i[:], 1.0)
            bc_dummy_o = persist.tile([2, 32], F32)
            nc.gpsimd.partition_broadcast(bc_dummy_o[:], bc_dummy_i[:])

            # ---- PE warm-up: dummy matmuls while input DMA streams, so
            # the PE pstate ramps to max before the real projections ----
            wrm_ps = psB.tile([128, 512], F32, tag="num0")
            for i in range(18):
                nc.tensor.matmul(
                    wrm_ps[:], lhsT=wrm_src[:, 0:128], rhs=wrm_src[:],
                    start=(i == 0), stop=(i == 17),
                )
            wrm_out = persist.tile([128, 512], BF16)
            nc.vector.tensor_copy(wrm_out[:], wrm_ps[:])

            qT = persist.tile([128, N], BF16)
            attnT = persist.tile([128, N], BF16)

            def qproj0():
                pps = psA.tile([128, NH], F32, tag="sim0", name="qp0")
                for isl in range(2):
                    for c in range(NC_DIM):
                        nc.tensor.matmul(
                            pps[:, ts(isl, 512)],
                            lhsT=wq[:, c, :],
                            rhs=nT[:, c, ts(isl, 512)],
                            start=(c == 0),
                            stop=(c == NC_DIM - 1),
                        )
                # ScalarE is idle until the first exp — bias lives there.
                nc.scalar.activation(
                    out=qT[:, 0:NH], in_=pps[:],
                    func=mybir.ActivationFunctionType.Identity, bias=bq[:],
                )

            def kproj_mm(half, pool, tag):
                kps = pool.tile([128, NH], F32, tag=tag, name=f"kp{half}")
                for isl in range(2):
                    for c in range(NC_DIM):
                        nc.tensor.matmul(
                            kps[:, ts(isl, 512)],
                            lhsT=wk[:, c, :],
                            rhs=nT[:, c, ts(half * 2 + isl, 512)],
                            start=(c == 0),
                            stop=(c == NC_DIM - 1),
                        )
                return kps

            def kproj_cast(half, kps, copy):
                # no k-bias: it only shifts each query row's logits
                # uniformly, which softmax cancels.
                copy(kTz[0:64, 0, ts(half, NH)], kps[0:64, :])
                copy(kTz[64:128, 1, ts(half, NH)], kps[64:128, :])

            sc_copy = lambda out_, in_: nc.scalar.copy(out=out_, in_=in_)

            # prologue projections: all four q/k halves, with every cast
            # and bias on ScalarE — idle until the first exp.  Keeping
            # these off DVE means the loop's DVE FIFO holds only
            # mask-muls and v-copies, so AVs never starve behind a
            # megacopy (starved PE trips the HAM clamp).
            qproj0()
            kps0 = kproj_mm(0, psA, "sim1")
            kproj_cast(0, kps0, sc_copy)
            kps1 = kproj_mm(1, psA, "sim1")
            kproj_cast(1, kps1, sc_copy)
            qps1 = psA.tile([128, NH], F32, tag="sim0", name="qp1")
            for isl in range(2):
                for c in range(NC_DIM):
                    nc.tensor.matmul(
                        qps1[:, ts(isl, 512)],
                        lhsT=wq[:, c, :],
                        rhs=nT[:, c, ts(2 + isl, 512)],
                        start=(c == 0),
                        stop=(c == NC_DIM - 1),
                    )
            nc.scalar.activation(
                out=qT[:, ts(1, NH)], in_=qps1[:],
                func=mybir.ActivationFunctionType.Identity, bias=bq[:],
            )

            def vproj(jb):
                vps = psA.tile([128, HG], F32, tag=f"sim{jb % 2}",
                               name=f"vp{jb}")
                for c in range(NC_DIM):
                    nc.tensor.matmul(
                        vps[:],
                        lhsT=nT[:, c, ts(jb, 128)],
                        rhs=wv[:, c, :],
                        start=(c == 0),
                        stop=(c == NC_DIM - 1),
                    )
                nc.vector.tensor_copy(
                    v_sb[:, jb, 0:130].rearrange("p (h c) -> p h c", h=2)[
                        :, :, 0:64
                    ],
                    vps[:].rearrange("p (h c) -> p h c", h=2),
                )

            def sim_exp_mask(jb, h, io):
                sps = psA.tile([128, NH], F32, tag=f"sim{h}", name="sps")
                for isl in range(2):
                    nc.tensor.matmul(
                        sps[:, ts(isl, 512)],
                        lhsT=kTz[:, h, ts(jb, 128)],
                        rhs=qT[:, ds(io + isl * 512, 512)],
                        start=True,
                        stop=True,
                    )
                pt = ptp.tile([128, NH], BF16, tag="pt")
                nc.scalar.activation(
                    out=pt[:],
                    in_=sps[:],
                    func=mybir.ActivationFunctionType.Exp,
                    scale=SCALE,
                )
                nc.vector.tensor_mul(pt[:], pt[:], maskT[:, jb, ds(io, NH)])
                return pt

            def av(jb, h, pt, npss):
                for isl in range(2):
                    nc.tensor.matmul(
                        npss[h][:, ts(isl, 512)],
                        lhsT=v_sb[:, jb, ts(h, 65)],
                        rhs=pt[:, ts(isl, 512)],
                        start=(jb == 0),
                        stop=(jb == NJB - 1),
                    )

            def norm_recip(ihalf, h, npss):
                if ihalf == 0:
                    # copy the accumulator to SBUF right away so the PSUM
                    # slot frees for i-half-1's AVs.
                    nsb = denp.tile([65, NH], F32, tag=f"nsb{h}")
                    nc.vector.tensor_copy(nsb[:], npss[h][:])
                    num_src = nsb[0:64, :]
                    # DVE silently reads partition 0 when its input AP
                    # starts at partition 64 — move the den row to a
                    # partition-0 tile (DMA on the idle gpsimd queue)
                    # before the reciprocal.
                    den1 = denp.tile([1, NH], F32, tag=f"den1{h}")
                    nc.gpsimd.dma_start(out=den1[:], in_=nsb[64:65, :])
                    den_src = den1[:]
                else:
                    # tail: nothing follows — read the accumulator in
                    # place; ScalarE is idle after the last exp.
                    num_src = npss[h][0:64, :]
                    den1 = denp.tile([1, NH], F32, tag=f"den1{h}")
                    nc.scalar.copy(out=den1[:], in_=npss[h][64:65, :])
                    den_src = den1[:]
                rec1 = denp.tile([1, NH], F32, tag=f"rec1{h}")
                nc.vector.reciprocal_approx_fast(out=rec1[:], in_=den_src)
                return num_src, rec1

            def norm_mul(h, io, num_src, rec1, copy):
                # replicate the reciprocal row across partitions 0:64 on
                # the Pool engine (library pre-loaded in the prologue) —
                # touches no PSUM slot, so the exp/sim stream never waits.
                rec = denp.tile([64, NH], F32, tag=f"rec{h}")
                nc.gpsimd.partition_broadcast(rec[:], rec1[:])
                nc.vector.tensor_mul(
                    attnT[ts(h, 64), ds(io, NH)], num_src, rec[:]
                )

            out_r = out_d
            osb = {}

            def oproj_ib(ib, copy):
                grp, k = ib // 4, ib % 4
                if k == 0:
                    osb[grp] = outp.tile([128, 4, DIM], BF16, tag="osb",
                                         name=f"osb{grp}")
                ops = psA.tile([128, DIM], F32, tag=f"sim{ib % 2}",
                               name=f"op{ib}")
                nc.tensor.matmul(
                    ops[:], lhsT=attnT[:, ts(ib, 128)], rhs=wo[:],
                    start=True, stop=True,
                )
                copy(osb[grp][:, k, :], ops[:])
                if k == 3:
                    eng = nc.scalar if grp == 3 else nc.gpsimd
                    eng.dma_start(
                        out=out_r[:, ts(grp, 4), :], in_=osb[grp][:]
                    )

            # ---- i-half 0: v-projection and the remaining q/k halves are
            # threaded into the loop's spare PE slots.  psB tag claim order
            # is warm -> kH1/qH1 -> AV accumulators, matching emission. ----
            io = 0
            npss0 = {
                0: psB.tile([65, NH], F32, tag="num0", name="nps00"),
                1: psB.tile([65, NH], F32, tag="num1", name="nps01"),
            }
            for jb in range(NJB):
                vproj(jb)
                for h in range(HEADS_PER_CORE):
                    pt = sim_exp_mask(jb, h, io)
                    av(jb, h, pt, npss0)
            norms0 = [norm_recip(0, h, npss0) for h in range(HEADS_PER_CORE)]

            # ---- i-half 1; i-half-0's norm-muls and o-projection are
            # spread through it so their PSUM aliasing (sim tags) costs at
            # most one slot-rotation bubble each ----
            io = NH
            npss1 = {
                0: psB.tile([65, NH], F32, tag="num0", name="nps10"),
                1: psB.tile([65, NH], F32, tag="num1", name="nps11"),
            }
            for jb in range(NJB):
                for h in range(HEADS_PER_CORE):
                    pt = sim_exp_mask(jb, h, io)
                    av(jb, h, pt, npss1)
                if 2 <= jb <= 3:
                    num_src, rec1 = norms0[jb - 2]
                    norm_mul(jb - 2, 0, num_src, rec1, nc.vector.tensor_copy)
                if 5 <= jb <= 12:
                    oproj_ib(jb - 5, nc.vector.tensor_copy)
            wrm2 = psA.tile([128, 512], F32, tag="sim0", name="wrm2")
            for i in range(14):
                nc.tensor.matmul(
                    wrm2[:], lhsT=wrm_src[:, 0:128], rhs=wrm_src[:],
                    start=(i == 0), stop=(i == 13),
                )
            for h in range(HEADS_PER_CORE):
                num_src, rec1 = norm_recip(1, h, npss1)
                norm_mul(h, io, num_src, rec1, sc_copy)
            # keep the PE hot until attnT is ready, then through the
            # output DMA — an idle PE trips the HAM clamp, which halves
            # the o-projection AND the final DMA rate.
            wrm3 = psA.tile([128, 512], F32, tag="sim1", name="wrm3")
            for i in range(20):
                nc.tensor.matmul(
                    wrm3[:], lhsT=wrm_src[:, 0:128], rhs=wrm_src[:],
                    start=(i == 0), stop=(i == 19),
                )
            for ib in range(8, 16):
                oproj_ib(ib, sc_copy)
            wrm4 = psB.tile([128, 512], F32, tag="num0", name="wrm4")
            for i in range(25):
                nc.tensor.matmul(
                    wrm4[:], lhsT=wrm_src[:, 0:128], rhs=wrm_src[:],
                    start=(i == 0), stop=(i == 24),
                )

    # Bacc.compile runs generate_event_semaphores, which splits multi-sem
    # waits down to the 1-wait-per-instruction limit walrus enforces.
    nc.compile()

    # Bacc's dce_regs leaves the (unread) engine-preamble register writes
    # behind at this kernel size, with deferred reg_id=-1 — walrus then
    # fails "Reg has not been allocated yet".  Nothing reads them, so any
    # valid unique per-engine id works.
    from collections import defaultdict

    next_id = defaultdict(lambda: 8)
    for a in nc.m.functions[0].allocations:
        if type(a).__name__ == "Register" and a.reg_id == -1:
            a.reg_id = next_id[str(a.engine)]
            next_id[str(a.engine)] += 1
    return nc


_NC_CACHE = None


def _get_nc():
    global _NC_CACHE
    if _NC_CACHE is None:
        _NC_CACHE = _build()
    return _NC_CACHE


def _prep_in_maps(nodes, edge_mask, wq, bq, wkv, bkv, wo, bo):
    bf16 = ml_dtypes.bfloat16
    wk_full, wv_full = wkv[:, :INNER], wkv[:, INNER:]
    per_batch = []
    for b in range(B):
        per_batch.append(
            (
                np.ascontiguousarray(nodes[b].T).astype(bf16),
                np.ascontiguousarray(edge_mask[b].T).astype(bf16),
            )
        )
    in_maps = []
    for core in range(NCORES):
        b, g = core // 4, core % 4
        cs = slice(g * HG, (g + 1) * HG)
        nT_b, maskT_b = per_batch[b]
        in_maps.append(
            {
                "nodesT": nT_b,
                "maskT": maskT_b,
                "wq_s": np.ascontiguousarray(wq[:, cs]).astype(bf16),
                "wk_s": np.ascontiguousarray(wk_full[:, cs]).astype(bf16),
                "wv_s": np.ascontiguousarray(wv_full[:, cs]).astype(bf16),
                "wo_s": np.ascontiguousarray(wo[cs, :]).astype(bf16),
                "bq_s": np.ascontiguousarray(bq[cs]).reshape(HG, 1).astype(np.float32),
            }
        )
    return in_maps


def kernel(nodes, edge_mask, wq, bq, wkv, bkv, wo, bo, _trace=False, _trace_kwargs=None):
    nodes = np.asarray(nodes, dtype=np.float32)
    edge_mask = np.asarray(edge_mask)
    wq = np.asarray(wq, dtype=np.float32)
    bq = np.asarray(bq, dtype=np.float32)
    wkv = np.asarray(wkv, dtype=np.float32)
    bkv = np.asarray(bkv, dtype=np.float32)
    wo = np.asarray(wo, dtype=np.float32)
    bo = np.asarray(bo, dtype=np.float32)

    nc = _get_nc()
    in_maps = _prep_in_maps(nodes, edge_mask, wq, bq, wkv, bkv, wo, bo)
    kw = {}
    if _trace:
        kw = dict(trace=True, **(_trace_kwargs or {}))
    res = run_bass_kernel_spmd(nc, in_maps, list(range(NCORES)), **kw)
    out = np.zeros((B, N, DIM), np.float32)
    for core in range(NCORES):
        o = res.results[core]["out"].astype(np.float32)   # [128, NJB, DIM]
        out[core // 4] += o.transpose(1, 0, 2).reshape(N, DIM)
    # v-bias shifts each head's attention output by exactly bv (softmax
    # weights sum to 1), so its output contribution is the constant bv @ wo.
    bv_full = bkv[INNER:]
    out += (bv_full @ wo + bo)[None, None, :]
    if _trace:
        return out, res
    return out
